# revision 1
# baseline (speedup 1.0000x reference)
"""DCNv2 (modulated deformable conv k=3 s=1 p=1) + BatchNorm(train) + ReLU on 8 TRN2 cores.

Sharding: data-parallel over batch (1 sample per core); BN statistics AllGather'd.

v2 pipeline (per core), engineered against the v1 instruction-cost model:
  - offset conv runs as float32r matmuls (1 cycle/col instead of f32's 4) in the
    slot-permuted column order; PSUM quadrants are scattered straight into the
    packed [36|36] map rows via partition-strided PSUM->SBUF DMAs (no DRAM bounce).
  - per-position math packs y and x into shared [100,1024] ops; the x0==-1
    pair-base swap is applied to BOTH halves (quad gather clamps y too);
    validity/idx chains run on GpSimd to shorten the DVE critical path.
  - a quad image xq (bf16 blocks [x[j], x[j+1], x[j+64], x[j+65]]) is built by 4
    casting gpsimd DMAs; ONE ap_gather per tap (int32 pairs, d=2) fetches all 4
    bilinear corners -- half the gather cost of bf16-element gathers.
  - per-tap coefficient quads are broadcast to 128 partitions from DRAM, split
    across the SP and ACT DMA queues; corner products on DVE (bf16 2x mode);
    the 4-way bilinear sum rides PE PSUM accumulation (stride-4 moving operand).
  - BN stats: Sum(x) on DVE + Sum(x^2) on ACT in parallel, AllGather (cheaper
    than AllReduce in the collective model) + local reduce, fused scale/bias+ReLU.
"""

import numpy as np
import ml_dtypes
from contextlib import ExitStack

import bass_rust
import concourse.bass as bass
import concourse.tile as tile
from concourse import bacc, mybir
from concourse.bass_utils import run_bass_kernel_spmd

F32 = mybir.dt.float32
F32R = mybir.dt.float32r
BF16 = mybir.dt.bfloat16
I32 = mybir.dt.int32
I16 = mybir.dt.int16
AF = mybir.ActivationFunctionType
ALU = mybir.AluOpType

B, CHI, CHO, H, W = 8, 128, 128, 64, 64
KK = 9
HW = H * W  # 4096
PADW = 66
NPAD = PADW * PADW  # 4356
EPS = 1e-5


def _ap(base, off, dims):
    """Custom AP rooted at an existing AP `base` (keeps symbolic tile tensor)."""
    return bass_rust.AP(base.tensor, base.offset + off, [list(d) for d in dims])


def build_kernel(n_cores=8):
    nc = bacc.Bacc("TRN2", target_bir_lowering=False, debug=False,
                   num_devices=n_cores)

    x_d = nc.dram_tensor("x", [CHI + 1, HW], F32, kind="ExternalInput")
    offw_d = nc.dram_tensor("offw", [KK, CHI, 27], F32, kind="ExternalInput")
    w_d = nc.dram_tensor("w", [KK, CHI, CHO], BF16, kind="ExternalInput")
    gridy_d = nc.dram_tensor("gridy", [100, 1024], F32, kind="ExternalInput")
    offbm_d = nc.dram_tensor("offbm", [36, 1], F32, kind="ExternalInput")
    gamma_d = nc.dram_tensor("gamma", [CHO], F32, kind="ExternalInput")
    beta_d = nc.dram_tensor("beta", [CHO], F32, kind="ExternalInput")
    out_d = nc.dram_tensor("out", [CHO, HW], F32, kind="ExternalOutput")

    with tile.TileContext(nc) as tc:
        with ExitStack() as ctx:
            _body(ctx, tc, nc, n_cores,
                  x_d, offw_d, w_d, gridy_d, offbm_d, gamma_d, beta_d,
                  out_d)
    nc.compile()
    return nc


def _body(ctx, tc, nc, n_cores,
          x_d, offw_d, w_d, gridy_d, offbm_d, gamma_d, beta_d, out_d):
    consts = ctx.enter_context(tc.tile_pool(name="consts", bufs=1))
    xqpool = ctx.enter_context(tc.tile_pool(name="xqpool", bufs=1))
    dram = ctx.enter_context(tc.tile_pool(name="dram", bufs=1, space="DRAM"))

    # ---- constant loads (ACT queue) -------------------------------------
    offw_sb = consts.tile([CHI, KK * 27], BF16)    # per tap t: cols 27t..27t+27
    nc.gpsimd.dma_start(offw_sb[:],
                        _ap(offw_d.ap(), 0, [[27, CHI], [CHI * 27, KK], [1, 27]]))
    w_sb = consts.tile([CHI, KK * CHO], BF16)
    nc.scalar.dma_start(w_sb[:],
                        _ap(w_d.ap(), 0, [[CHO, CHI], [CHI * CHO, KK], [1, CHO]]))
    gridy = consts.tile([100, 1024], F32)
    nc.scalar.dma_start(gridy[:], gridy_d.ap())
    offbm = consts.tile([36, 1], F32)
    nc.scalar.dma_start(offbm[:], offbm_d.ap())
    gam = consts.tile([CHO, 1], F32)
    nc.scalar.dma_start(gam[:], _ap(gamma_d.ap(), 0, [[1, CHO], [1, 1]]))
    bet = consts.tile([CHO, 1], F32)
    nc.scalar.dma_start(bet[:], _ap(beta_d.ap(), 0, [[1, CHO], [1, 1]]))

    # pair image PA[c, j] = bf16 pair (x[c,j], x[c,j+1]) for j in [0, 4160):
    # rows 0..64 of the padded image, so idx+64 fetches the bottom corner row.
    NPA = HW + 64
    pa = xqpool.tile([CHI, NPA], I32)
    pab = pa[:].bitcast(BF16)
    pabs = pab.ap[0][0]
    # coefficient pair-tiles + gather base indices; reserved up front so their
    # addresses never overlap the scoped maps pool (they are read in phase 3)
    cqT = xqpool.tile([36, 2 * 1024], BF16, tag="cqT", name="cqT")
    cqB = xqpool.tile([36, 2 * 1024], BF16, tag="cqB", name="cqB")
    cqTs = cqT[:].ap[0][0]
    cqBs = cqB[:].ap[0][0]
    ii = xqpool.tile([36, 1024], I16, tag="ii", name="ii")
    iis = ii[:].ap[0][0]
    iib = xqpool.tile([36, 1024], I16, tag="iib", name="iib")
    iibs = iib[:].ap[0][0]
    # liveness anchors: keep the allocator from aliasing these over scoped
    # maps tiles (their real writes are scheduled mid-kernel)
    nc.vector.memset(cqT[:, 0:1], 0.0)
    nc.vector.memset(cqB[:, 0:1], 0.0)
    nc.vector.memset(ii[:, 0:1], 0)
    nc.vector.memset(iib[:, 0:1], 0)

    # ---- DRAM scratch ----------------------------------------------------
    idram = dram.tile([KK, 2 * HW], I16)
    cdram = dram.tile([KK, 4 * HW], BF16)
    cc_in = dram.tile([CHO, 2], F32)
    cc_out = dram.tile([n_cores, CHO * 2], F32)

    # ---- scoped: pad image, offset conv, per-position maps --------------
    with tc.tile_pool(name="maps", bufs=1) as maps, \
         tc.tile_pool(name="pads", bufs=1) as pads:
        xpad = pads.tile([CHI, NPAD], BF16)
        oyx = maps.tile([100, 1024], F32, tag="oyx")
        mk = maps.tile([36, 1024], F32, tag="mk")
        xps = xpad[:].ap[0][0]
        oys = oyx[:].ap[0][0]
        mks = mk[:].ap[0][0]

        # zero only the 1-pixel pad border; interior is overwritten
        nc.vector.memset(_ap(xpad[:], 0, [[xps, CHI], [1, PADW]]), 0.0)
        nc.vector.memset(_ap(xpad[:], 65 * PADW, [[xps, CHI], [1, PADW]]), 0.0)
        nc.vector.memset(
            _ap(xpad[:], PADW, [[xps, CHI], [PADW, 64], [1, 1]]), 0.0)
        nc.vector.memset(
            _ap(xpad[:], PADW + 65, [[xps, CHI], [PADW, 64], [1, 1]]), 0.0)
        # interior: pad[(y+1)*66 + (x+1)] = bf16(x[y*64 + x]) (casting gpsimd DMA)
        nc.gpsimd.dma_start(
            _ap(xpad[:], PADW + 1, [[xps, CHI], [PADW, H], [1, W]]),
            _ap(x_d.ap(), 0, [[HW, CHI], [W, H], [1, W]]))

        # pair image from xpad (rows 0..64; row 64 = pad zeros). Two DVE
        # 4x-mode copies: even-j pairs and odd-j pairs.
        for par in range(2):
            nc.vector.tensor_copy(
                _ap(pab, 2 * par, [[pabs, CHI], [128, 65], [4, 32], [1, 2]]),
                _ap(xpad[:], PADW + 1 + par,
                    [[xps, CHI], [PADW, 65], [2, 32], [1, 2]]))

        # PE warm-up: junk matmuls keep the ramp model hot until xpad lands
        with tc.tile_pool(name="warmps", bufs=1, space="PSUM") as wps:
            wj = wps.tile([27, 243], F32)
            for i in range(26):
                nc.tensor.matmul(wj[:], offw_sb[:, 0:27], offw_sb[:, 0:243],
                                 start=(i == 0), stop=(i == 25))

        # ---- offset conv (slot-ordered columns), bf16 matmuls ----------
        # psum rows 0:9 = y offsets, 9:18 = x offsets, 18:27 = mask logits;
        # quadrant q bounces once through om_dram; 3 packed readbacks land in
        # the row-(4k+q) map layout (y rows 0:36, x rows 64:100, mask in mk).
        om_dram = dram.tile([27, 4096], F32)
        with tc.tile_pool(name="ompsum", bufs=2, space="PSUM") as omp:
            qdma = [nc.sync, nc.scalar, nc.sync, nc.scalar]
            for q in range(4):
                om_ps = omp.tile([27, 1024], F32, tag="om")
                for t in range(KK):
                    di, dj = t // 3, t % 3
                    for h2 in range(2):
                        # column c in [512*h2, 512*h2+512): y = 4*(c%16)+q, x = c//16
                        rhs = _ap(xpad[:], (q + di) * PADW + 32 * h2 + dj,
                                  [[xps, CHI], [1, 32], [4 * PADW, 16]])
                        nc.tensor.matmul(
                            om_ps[:, 512 * h2:512 * h2 + 512],
                            offw_sb[:, 27 * t:27 * t + 27],
                            rhs, start=(t == 0), stop=(t == KK - 1))
                om_sb = maps.tile([27, 1024], F32, tag="om_sb", name="om_sb",
                                  bufs=2)
                if q % 2 == 0:
                    nc.scalar.activation(om_sb[:], om_ps[:], AF.Copy)
                else:
                    nc.vector.tensor_copy(om_sb[:], om_ps[:])
                oms = om_sb[:].ap[0][0]
                qdma[q].dma_start(
                    _ap(om_dram[:], q * 1024, [[4096, 27], [1, 1024]]),
                    _ap(om_sb[:], 0, [[oms, 27], [1, 1024]]))
            nc.vector.memset(oyx[32:64, :], 0.0)   # unused gap rows
            nc.sync.dma_start(
                oyx[0:36, :],
                _ap(om_dram[:], 0, [[4096, KK], [1024, 4], [1, 1024]]))
            nc.scalar.dma_start(
                oyx[64:100, :],
                _ap(om_dram[:], 9 * 4096, [[4096, KK], [1024, 4], [1, 1024]]))
            nc.gpsimd.dma_start(
                mk[:],
                _ap(om_dram[:], 18 * 4096, [[4096, KK], [1024, 4], [1, 1024]]))

        # ---- per-position math on [100,1024] packed maps --------------
        ts_ = nc.vector.tensor_scalar
        tt = nc.vector.tensor_tensor
        stt = nc.vector.scalar_tensor_tensor
        cp = nc.vector.tensor_copy

        def T2(tag, dt=F32):
            return maps.tile([100, 1024], dt, tag=tag, name=tag)

        def T(tag, dt=F32):
            return maps.tile([36, 1024], dt, tag=tag, name=tag)

        pyx = oyx                              # in-place add
        tt(pyx[:], oyx[:], gridy[:], ALU.add)
        # floor() robust to the convert rounding mode (HW: RNE, sim: trunc)
        ti = T2("u1", I32)
        cp(ti[:], pyx[:])
        fyx = T2("u2")
        cp(fyx[:], ti[:])
        gg = T2("u1b")
        tt(gg[:], fyx[:], pyx[:], ALU.is_gt)
        tt(fyx[:], fyx[:], gg[:], ALU.subtract)
        # ---- base-index chain FIRST (it gates the first gather) ----------
        yc = T("t4b"); ts_(yc[:], fyx[0:36, :], 0.0, 63.0, ALU.max, ALU.min)
        xc = T2("u1c")
        ts_(xc[64:100, :], fyx[64:100, :], 0.0, 63.0, ALU.max, ALU.min)
        xcl = T("t1"); nc.scalar.dma_start(xcl[:], xc[64:100, :])
        sig = T("sg", BF16)
        nc.scalar.activation(sig[:], mk[:], AF.Sigmoid, bias=offbm[:])
        # weights in bf16 (integers <= 64 and [0,1] weights are exact/ample;
        # TensorScalar ops ride the 4x mode, TensorTensor the 2x mode)
        fyb = T2("b0", BF16); cp(fyb[:], fyx[:])
        lyx = T2("b1", BF16); tt(lyx[:], pyx[:], fyx[:], ALU.subtract)
        pi = T("t2"); stt(pi[:], yc[:], float(W), xcl[:], ALU.mult, ALU.add)
        cp(ii[:], pi[:])
        ts_(iib[:], pi[:], 64.0, None, ALU.add)
        # idram writes for taps 0/1 as soon as the indices exist
        for k in range(2):
            qd = nc.sync if k % 2 == 0 else nc.scalar
            qd.dma_start(
                _ap(idram[:], k * 2 * HW, [[64, 4], [1, 64], [256, 16]]),
                _ap(ii[:], 4 * k * iis, [[iis, 4], [16, 64], [1, 16]]))
            qd.dma_start(
                _ap(idram[:], k * 2 * HW + HW, [[64, 4], [1, 64], [256, 16]]),
                _ap(iib[:], 4 * k * iibs, [[iibs, 4], [16, 64], [1, 16]]))
        # ---- corner weights ----------------------------------------------
        myx = T2("b2", BF16); ts_(myx[:], lyx[:], -1.0, 1.0, ALU.mult, ALU.add)
        ca = T2("b3", BF16); ts_(ca[:], fyb[:], 0.0, 63.0, ALU.max, ALU.min)
        vtl = T2("b4", BF16); tt(vtl[:], ca[:], fyb[:], ALU.is_equal)
        cb2 = T2("b3b", BF16); ts_(cb2[:], fyb[:], -1.0, 62.0, ALU.max, ALU.min)
        vbr = T2("b4b", BF16); tt(vbr[:], cb2[:], fyb[:], ALU.is_equal)
        wA = T2("b5", BF16); tt(wA[:], myx[:], vtl[:], ALU.mult)
        wB = T2("b6", BF16); tt(wB[:], lyx[:], vbr[:], ALU.mult)
        # f == -1 quad-base swap, both halves (quad clamps y AND x bases)
        sl = T2("b7", BF16)
        stt(sl[:], fyb[:], -1.0, wB[:], ALU.is_equal, ALU.mult)
        tt(wA[:], wA[:], sl[:], ALU.add)
        tt(wB[:], wB[:], sl[:], ALU.subtract)
        # bring x halves onto partitions 0:36 (cross-partition -> DMA)
        wxL = T("t8", BF16); nc.gpsimd.dma_start(wxL[:], wA[64:100, :])
        wxR = T("t9", BF16); nc.sync.dma_start(wxR[:], wB[64:100, :])
        # mask fold into the x halves
        tt(wxL[:], wxL[:], sig[:], ALU.mult)
        tt(wxR[:], wxR[:], sig[:], ALU.mult)
        # coefficient pair tiles [36, 2048] bf16 in gather-position order:
        # row elem E = 128*b + 2*a + c01 for map column c = 16*a + b;
        # cqT holds (TL,TR), cqB holds (BL,BR).
        for (cqt, cts), wy in (((cqT, cqTs), wA), ((cqB, cqBs), wB)):
            for c01, wx in enumerate((wxL, wxR)):
                wys = wy[:].ap[0][0]
                wxs = wx[:].ap[0][0]
                tt(_ap(cqt[:], c01, [[cts, 36], [2, 16], [32, 64]]),
                   _ap(wy[:], 0, [[wys, 36], [1, 16], [16, 64]]),
                   _ap(wx[:], 0, [[wxs, 36], [1, 16], [16, 64]]),
                   ALU.mult)

        # coef writes for taps 0/1
        for k in range(2):
            qd = nc.sync if k % 2 == 0 else nc.scalar
            qd.dma_start(
                _ap(cdram[:], k * 4 * HW, [[2048, 4], [1, 2048]]),
                _ap(cqT[:], 4 * k * cqTs, [[cqTs, 4], [1, 2048]]))
            qd.dma_start(
                _ap(cdram[:], k * 4 * HW + 2 * HW, [[2048, 4], [1, 2048]]),
                _ap(cqB[:], 4 * k * cqBs, [[cqBs, 4], [1, 2048]]))

    # ---- gather + interp + main conv (one 8192-idx gather per tap) ------
    # gather pos i = 4096*s + 2048*h + i_loc, i_loc = 512*q + 64*b'' + a
    # (slot col c = 16a+b, b = 8s+b''); h=0 top pairs (idx), h=1 bottom (+64).
    gpool = ctx.enter_context(tc.tile_pool(name="gpool", bufs=2))
    out_pp = ctx.enter_context(tc.tile_pool(name="outp", bufs=1, space="PSUM"))
    out_ps = out_pp.tile([CHO, HW], F32)
    bn = ctx.enter_context(tc.tile_pool(name="bn", bufs=1))
    zerob = bn.tile([CHO, 1], F32)
    nc.vector.memset(zerob[:], 0.0)
    p1 = bn.tile([CHO, 4], F32)
    p2 = bn.tile([CHO, 4], F32)
    tt = nc.vector.tensor_tensor
    cp = nc.vector.tensor_copy
    ts_ = nc.vector.tensor_scalar

    staged = 2
    for k in range(KK):
        if k == 1:
            # preload the Sqrt/Relu activation tables off the critical path
            warm = bn.tile([CHO, 1], F32, tag="warm", name="warm")
            nc.scalar.activation(warm[:], zerob[:], AF.Sqrt, bias=zerob[:])
            nc.scalar.activation(warm[:], zerob[:], AF.Relu)
        # idx: top + bottom halves from DRAM (wrapped)
        ix = gpool.tile([128, 512], I16, tag="ix", name="ix", bufs=3)
        nc.gpsimd.dma_start(
            ix[:, 0:256],
            _ap(idram[:], k * 2 * HW, [[0, 8], [256, 16], [1, 256]]))
        nc.gpsimd.dma_start(
            ix[:, 256:512],
            _ap(idram[:], k * 2 * HW + HW, [[0, 8], [256, 16], [1, 256]]))
        g = gpool.tile([128, 2 * HW], I32, tag="g", name="g", bufs=3)
        nc.gpsimd.ap_gather(g[:], pa[:], ix[:], channels=128,
                            num_elems=NPA, d=1, num_idxs=2 * HW)
        gb = g[:].bitcast(BF16)   # [128, 16384]
        gbs = gb.ap[0][0]
        for h in range(2):
            cb = gpool.tile([128, 2 * HW], BF16, tag="cb", name="cb", bufs=3)
            (nc.sync if h == 0 else nc.scalar).dma_start(
                cb[:, 0:HW],
                _ap(cdram[:], (k * 4 + 2 * h) * HW, [[0, 128], [1, HW]]))
            (nc.scalar if h == 0 else nc.sync).dma_start(
                cb[:, HW:2 * HW],
                _ap(cdram[:], (k * 4 + 2 * h) * HW + HW, [[0, 128], [1, HW]]))
            gh = _ap(gb, 8192 * h, [[gbs, 128], [1, 8192]])
            tt(gh, cb[:], gh, ALU.mult)
            for c8 in range(8):
                for c01 in range(2):
                    # psum col 256u+64q+a <- g elem 8192h+2048q+32a+4c8+2u+c01
                    rhs = _ap(gb, 8192 * h + 4 * c8 + c01,
                              [[gbs, 128], [2, 2], [2048, 4], [32, 64]])
                    nc.tensor.matmul(
                        out_ps[:, 512 * c8:512 * c8 + 512],
                        w_sb[:, CHO * k:CHO * k + CHO],
                        rhs, start=(k == 0 and h == 0 and c01 == 0),
                        stop=(k == KK - 1 and h == 1 and c01 == 1))
                if k == KK - 1 and h == 1 and c8 % 2 == 1:
                    # 1024-col chunk complete: BN partials chase the last tap
                    c4 = c8 // 2
                    sl8 = slice(1024 * c4, 1024 * c4 + 1024)
                    stg = bn.tile([CHO, 1024], F32, tag="stg", name="stg",
                                  bufs=3)
                    nc.scalar.activation(stg[:], out_ps[:, sl8],
                                         AF.Square, bias=zerob[:],
                                         accum_out=p2[:, c4:c4 + 1])
                    nc.vector.tensor_reduce(p1[:, c4:c4 + 1], out_ps[:, sl8],
                                            mybir.AxisListType.X, ALU.add)
        # stage the (k+2)'th tap's idx/coef DRAM writes behind this tap's DMAs
        if staged < KK:
            kk = staged
            qa = nc.sync if kk % 2 == 0 else nc.scalar
            qb = nc.scalar if kk % 2 == 0 else nc.sync
            qa.dma_start(
                _ap(idram[:], kk * 2 * HW, [[64, 4], [1, 64], [256, 16]]),
                _ap(ii[:], 4 * kk * iis, [[iis, 4], [16, 64], [1, 16]]))
            qb.dma_start(
                _ap(idram[:], kk * 2 * HW + HW, [[64, 4], [1, 64], [256, 16]]),
                _ap(iib[:], 4 * kk * iibs, [[iibs, 4], [16, 64], [1, 16]]))
            qa.dma_start(
                _ap(cdram[:], kk * 4 * HW, [[2048, 4], [1, 2048]]),
                _ap(cqT[:], 4 * kk * cqTs, [[cqTs, 4], [1, 2048]]))
            qb.dma_start(
                _ap(cdram[:], kk * 4 * HW + 2 * HW, [[2048, 4], [1, 2048]]),
                _ap(cqB[:], 4 * kk * cqBs, [[cqBs, 4], [1, 2048]]))
            staged += 1

    # ---- BatchNorm (AllGather'd stats) + ReLU ---------------------------
    ccs = bn.tile([CHO, 2], F32)
    nc.vector.tensor_reduce(ccs[:, 0:1], p1[:], mybir.AxisListType.X, ALU.add)
    nc.vector.tensor_reduce(ccs[:, 1:2], p2[:], mybir.AxisListType.X, ALU.add)
    nc.sync.dma_start(cc_in[:], ccs[:])
    nc.gpsimd.collective_compute(
        "AllGather", ALU.bypass, replica_groups=[list(range(n_cores))],
        ins=[cc_in.opt()], outs=[cc_out.opt()])
    st = bn.tile([CHO, 2 * n_cores], F32)
    nc.sync.dma_start(
        st[:], _ap(cc_out[:], 0, [[2, CHO], [CHO * 2, n_cores], [1, 2]]))
    sts = st[:].ap[0][0]
    ss = bn.tile([CHO, 2], F32)
    nc.vector.tensor_reduce(
        ss[:], _ap(st[:], 0, [[sts, CHO], [1, 2], [2, n_cores]]),
        mybir.AxisListType.X, ALU.add)
    inv = 1.0 / float(n_cores * HW)
    mu = bn.tile([CHO, 1], F32); ts_(mu[:], ss[:, 0:1], inv, None, ALU.mult)
    ex2 = bn.tile([CHO, 1], F32); ts_(ex2[:], ss[:, 1:2], inv, None, ALU.mult)
    m2 = bn.tile([CHO, 1], F32); tt(m2[:], mu[:], mu[:], ALU.mult)
    var = bn.tile([CHO, 1], F32); tt(var[:], ex2[:], m2[:], ALU.subtract)
    epsb = bn.tile([CHO, 1], F32)
    nc.vector.memset(epsb[:], EPS)
    sd = bn.tile([CHO, 1], F32)
    nc.scalar.activation(sd[:], var[:], AF.Sqrt, bias=epsb[:])
    rsd = bn.tile([CHO, 1], F32)
    nc.vector.reciprocal(rsd[:], sd[:])
    sc = bn.tile([CHO, 1], F32); tt(sc[:], rsd[:], gam[:], ALU.mult)
    msc = bn.tile([CHO, 1], F32); tt(msc[:], mu[:], sc[:], ALU.mult)
    bb = bn.tile([CHO, 1], F32); tt(bb[:], bet[:], msc[:], ALU.subtract)
    for c8 in range(8):
        sl8 = slice(512 * c8, 512 * c8 + 512)
        stg = bn.tile([CHO, 1024], F32, tag="stg", name="stg", bufs=3)
        if c8 < 5:
            nc.scalar.activation(stg[:, 0:512], out_ps[:, sl8], AF.Relu,
                                 bias=bb[:], scale=sc[:])
        else:
            ts_(stg[:, 0:512], out_ps[:, sl8], sc[:], bb[:],
                ALU.mult, ALU.add)
            ts_(stg[:, 0:512], stg[:, 0:512], 0.0, None, ALU.max)
        (nc.sync if c8 % 2 == 0 else nc.gpsimd).dma_start(
            _ap(out_d.ap(), 512 * c8, [[HW, CHO], [1, 512]]),
            stg[:, 0:512])


# ---------------- host side ----------------------------------------------

_PERM = [2 * k for k in range(KK)] + [2 * k + 1 for k in range(KK)] + \
        [2 * KK + k for k in range(KK)]


def host_inputs(x, off_w, off_b, w, b, gamma, beta):
    """Per-core input maps (core i gets sample i)."""
    x = np.asarray(x, np.float32)
    off_w = np.asarray(off_w, np.float32)
    off_b = np.asarray(off_b, np.float32)
    w = np.asarray(w, np.float32)
    gamma = np.asarray(gamma, np.float32)
    beta = np.asarray(beta, np.float32)

    offw_r = off_w[_PERM]                                   # [27,128,3,3]
    offw_t = np.ascontiguousarray(
        offw_r.reshape(27, CHI, 9).transpose(2, 1, 0))      # [9,128,27]
    offb_r = off_b[_PERM]
    w_t = np.ascontiguousarray(
        w.reshape(CHO, CHI, 9).transpose(2, 1, 0)).astype(ml_dtypes.bfloat16)

    q = np.arange(4)[:, None, None]          # chunk
    k = np.arange(KK)[None, :, None]         # tap
    c = np.arange(1024)[None, None, :]       # col
    ymap = 4.0 * (c % 16) + q                # y of slot
    xmap = c // 16                           # x of slot
    gridy_h = np.ascontiguousarray(np.broadcast_to(
        ymap - 1.0 + k // 3 + offb_r[:KK][None, :, None],
        (4, KK, 1024)).transpose(1, 0, 2)).reshape(36, 1024)
    gridx_h = np.ascontiguousarray(np.broadcast_to(
        xmap - 1.0 + k % 3 + offb_r[KK:2 * KK][None, :, None],
        (4, KK, 1024)).transpose(1, 0, 2)).reshape(36, 1024)
    gridy = np.zeros((100, 1024), np.float32)
    gridy[0:36] = gridy_h
    gridy[64:100] = gridx_h
    offbm = np.repeat(offb_r[2 * KK:], 4).reshape(36, 1)

    shared = {
        "offw": offw_t.astype(np.float32),
        "w": w_t,
        "gridy": np.ascontiguousarray(gridy, np.float32),
        "offbm": np.ascontiguousarray(offbm, np.float32),
        "gamma": gamma, "beta": beta,
    }
    zrow = np.zeros((1, HW), np.float32)
    return [dict(shared,
                 x=np.ascontiguousarray(
                     np.concatenate([x[i].reshape(CHI, HW), zrow], axis=0)))
            for i in range(B)]


_NC_CACHE = {}


def _get_nc(n_cores=8):
    if n_cores not in _NC_CACHE:
        _NC_CACHE[n_cores] = build_kernel(n_cores)
    return _NC_CACHE[n_cores]


def kernel(x, off_w, off_b, w, b, gamma, beta):
    nc = _get_nc(8)
    in_maps = host_inputs(x, off_w, off_b, w, b, gamma, beta)
    res = None
    for attempt in range(3):
        try:
            res = run_bass_kernel_spmd(nc, in_maps, core_ids=list(range(8)))
            break
        except Exception:
            # a crashed prior session can leave a core in
            # NRT_EXEC_UNIT_UNRECOVERABLE; a fresh attempt resets it
            if attempt == 2:
                raise
    out = np.stack([res.results[i]["out"] for i in range(8)], axis=0)
    return out.reshape(B, CHO, H, W).astype(np.float32)

